# revision 17
# baseline (speedup 1.0000x reference)
"""CAM-module kernel for Trainium2, 8 NeuronCores, data-parallel over batch.

Per batch b (B=16, C=512, N=H*W=4096), with Q_b = x[b] reshaped (N, C):
    E_b   = Q_b^T Q_b                      (C x C gram, fp32r matmuls)
    mx[d] = max_c E_0[c, d]                (from batch 0 ONLY; E symmetric ->
                                            row-max of E_0 works)
    A_b   = softmax(mx - E_b, axis=-1)
    out_b = gamma * (A_b @ Q_b^T) + x[b]

Sharding: core i handles batches (i, i+8); every core redundantly computes
E_0 (from a shared x0 input) to obtain mx without collectives.

Layout trick: Q is stored in SBUF as stride-8 row-interleaved tiles
qs[(k,g)][p, f] = Q[1024*g + 8*p + k, f]. These serve as (a) mm1 contraction
chunks (sum order irrelevant), and (b) the epilogue "+x" operand: the x
values needed for output tile [c-chunk g, n-chunk k] are exactly qs[(k,g)].
Q^T for mm2 comes from DMA-transposed loads of a host-precast bf16 copy.
"""

import numpy as np
import ml_dtypes

B, C, HW = 16, 512, 4096
NCORES = 8
BPC = 2  # batches per core
KC = 8   # n-chunk count (512-wide chunks of HW)
G = 4    # c-chunk count (128-wide chunks of C)
NCH = 32  # mm1 contraction chunks (of 128)

_cache = {}


def _build_nc(stage: int = 4):
    """stage: 1=phase0 only, 2=+mm1+softmax, 3=+transposes, 4=full."""
    import concourse.tile as tile
    from concourse import bacc, mybir
    from concourse.masks import make_identity

    f32 = mybir.dt.float32
    f32r = mybir.dt.float32r
    bf16 = mybir.dt.bfloat16
    AluOp = mybir.AluOpType
    ActFn = mybir.ActivationFunctionType

    nc = bacc.Bacc("TRN2", target_bir_lowering=False, debug=False,
                   num_devices=NCORES)

    xq = nc.dram_tensor("xq", [BPC, HW, C], f32r, kind="ExternalInput")
    x0q = nc.dram_tensor("x0q", [HW, C], f32r, kind="ExternalInput")
    # host-pretransposed bf16 copy: qt16[j] = Q_j^T  (C x HW)
    qt16 = nc.dram_tensor("qt16", [BPC, C, HW], bf16, kind="ExternalInput")
    gamma = nc.dram_tensor("gamma", [1, 1], f32, kind="ExternalInput")
    out = nc.dram_tensor("out", [BPC, C, HW], f32, kind="ExternalOutput")

    with tile.TileContext(nc) as tc:
        with (
            tc.tile_pool(name="consts", bufs=1) as consts,
            tc.tile_pool(name="qs", bufs=40) as qsp,
            tc.tile_pool(name="qt", bufs=34) as qtp,
            tc.tile_pool(name="pp", bufs=8) as ppp,
            tc.tile_pool(name="pt", bufs=18) as ptp,
            tc.tile_pool(name="e2", bufs=5) as e2p,
            tc.tile_pool(name="res", bufs=6) as resp,
            tc.tile_pool(name="small", bufs=10) as smallp,
            tc.tile_pool(name="eps", bufs=4, space="PSUM") as epsp,
            tc.tile_pool(name="ops", bufs=4, space="PSUM") as opsp,
        ):
            # ---- constants
            ident = consts.tile([128, 128], f32, name="ident")
            make_identity(nc, ident[:])
            ident_bf = consts.tile([128, 128], bf16, name="ident_bf")
            make_identity(nc, ident_bf[:])
            ones1 = consts.tile([1, 128], f32, name="ones1")
            nc.vector.memset(ones1[:], 1.0)
            gb = consts.tile([128, 1], f32, name="gb")
            nc.gpsimd.dma_start(out=gb[:], in_=gamma.ap().to_broadcast([128, 1]))
            mxb = consts.tile([128, C], f32, name="mxb")

            # ---- phase 0: mx from x0q (E0 = Q0^T Q0, mx = row-max by symmetry)
            e0 = [epsp.tile([128, C], f32, name=f"e0_{c0}", tag="e")
                  for c0 in range(G)]
            x0r = x0q.ap().rearrange("(t p) c -> t p c", p=128)
            for t in range(NCH):
                q0 = qsp.tile([128, C], f32r, name=f"q0_{t}", tag="qs")
                nc.sync.dma_start(out=q0[:], in_=x0r[t])
                for c0 in range(G):
                    nc.tensor.matmul(
                        e0[c0][:],
                        lhsT=q0[:, c0 * 128:(c0 + 1) * 128],
                        rhs=q0[:],
                        start=(t == 0),
                        stop=(t == NCH - 1),
                    )
            mxv = [smallp.tile([128, 1], f32, name=f"mxv_{c0}", tag="mxv")
                   for c0 in range(G)]
            for c0 in range(G):
                nc.vector.reduce_max(out=mxv[c0][:], in_=e0[c0][:],
                                     axis=mybir.AxisListType.X)
            mxrow_ps = opsp.tile([1, C], f32, name="mxrow_ps", tag="o")
            for c0 in range(G):
                nc.tensor.matmul(
                    mxrow_ps[:, c0 * 128:(c0 + 1) * 128],
                    lhsT=mxv[c0][:],
                    rhs=ident[:],
                    start=True,
                    stop=True,
                )
            mxrow = smallp.tile([1, C], f32, name="mxrow", tag="mxrow")
            nc.vector.tensor_copy(mxrow[:], mxrow_ps[:])
            mxb_ps = opsp.tile([128, C], f32, name="mxb_ps", tag="o")
            nc.tensor.matmul(mxb_ps[:], lhsT=ones1[:], rhs=mxrow[:],
                             start=True, stop=True)
            nc.vector.tensor_copy(mxb[:], mxb_ps[:])

            if stage <= 1:
                dbg = resp.tile([128, C], f32, name="dbg", tag="res")
                nc.vector.tensor_copy(dbg[:], mxb[:])
                nc.sync.dma_start(out=out.ap()[0][0:128, 0:C], in_=dbg[:])
                nbatch = 0
            else:
                nbatch = BPC

            # ---- per-batch pipeline
            for b in range(nbatch):
                xq_b = xq.ap()[b].rearrange("(g p k) c -> k g p c", k=KC, p=128)
                qt16_b = qt16.ap()[b]
                out_b = out.ap()[b]

                # Q resident tiles (stride-8 interleaved)
                qs = {}
                for k in range(KC):
                    for g in range(G):
                        qt_ = qsp.tile([128, C], f32r, name=f"qs{b}_{k}_{g}",
                                       tag="qs")
                        nc.sync.dma_start(out=qt_[:], in_=xq_b[k, g])
                        qs[(k, g)] = qt_

                # Q^T bf16 tiles: plain loads of the host-pretransposed copy
                QT = {}
                if stage in (3, 4):
                    for d0 in range(G):
                        for n0 in range(KC):
                            t_ = qtp.tile([128, 512], bf16,
                                          name=f"qt{b}_{d0}_{n0}", tag="qt")
                            nc.sync.dma_start(
                                out=t_[:],
                                in_=qt16_b[d0 * 128:(d0 + 1) * 128,
                                           n0 * 512:(n0 + 1) * 512],
                            )
                            QT[(d0, n0)] = t_

                # mm1: E = Q^T Q, fp32r, accumulate over 32 chunks
                e = [epsp.tile([128, C], f32, name=f"e{b}_{c0}", tag="e")
                     for c0 in range(G)]
                ci = 0
                for k in range(KC):
                    for g in range(G):
                        q = qs[(k, g)]
                        for c0 in range(G):
                            nc.tensor.matmul(
                                e[c0][:],
                                lhsT=q[:, c0 * 128:(c0 + 1) * 128],
                                rhs=q[:],
                                start=(ci == 0),
                                stop=(ci == NCH - 1),
                            )
                        ci += 1

                if stage == 12:
                    for c0 in range(G):
                        dbge = resp.tile([128, C], f32, name=f"dbge{b}_{c0}",
                                         tag="res")
                        nc.vector.tensor_copy(dbge[:], e[c0][:])
                        nc.sync.dma_start(
                            out=out_b[c0 * 128:(c0 + 1) * 128, 0:C],
                            in_=dbge[:],
                        )
                    continue

                # softmax (unnormalized): P = exp((mx - E) - rowmax(mx - E))
                # computed via e2 = E - mx ; m2 = min(e2) ; P = exp(-e2 + m2)
                P = []
                gR = []
                for c0 in range(G):
                    e2 = e2p.tile([128, C], f32, name=f"e2{b}_{c0}", tag="e2")
                    m2 = smallp.tile([128, 1], f32, name=f"m2{b}_{c0}", tag="m2")
                    nc.vector.tensor_sub(e2[:], e[c0][:], mxb[:])
                    nc.vector.tensor_reduce(
                        out=m2[:], in_=e2[:], axis=mybir.AxisListType.X,
                        op=AluOp.min,
                    )
                    if stage == 13:
                        dbg2 = resp.tile([128, C], f32, name=f"dbg2{b}_{c0}",
                                         tag="res")
                        nc.vector.tensor_copy(dbg2[:], e2[:])
                        nc.sync.dma_start(
                            out=out_b[c0 * 128:(c0 + 1) * 128, 0:C],
                            in_=dbg2[:],
                        )
                        continue
                    p_ = ppp.tile([128, C], bf16, name=f"p{b}_{c0}", tag="p")
                    z = smallp.tile([128, 1], f32, name=f"z{b}_{c0}", tag="z")
                    nc.scalar.activation(
                        out=p_[:],
                        in_=e2[:],
                        func=ActFn.Exp,
                        bias=m2[:],
                        scale=-1.0,
                        accum_out=z[:],
                    )
                    r_ = smallp.tile([128, 1], f32, name=f"r{b}_{c0}", tag="r")
                    nc.vector.reciprocal(r_[:], z[:])
                    gr = smallp.tile([128, 1], f32, name=f"gr{b}_{c0}", tag="gr")
                    nc.vector.tensor_mul(gr[:], r_[:], gb[:])
                    P.append(p_)
                    gR.append(gr)
                if stage == 13:
                    continue

                if stage <= 2:
                    # dump P (cast to f32) so the stage is observable
                    for c0 in range(G):
                        dbgp = resp.tile([128, C], f32, name=f"dbgp{b}_{c0}",
                                         tag="res")
                        nc.vector.tensor_copy(dbgp[:], P[c0][:])
                        nc.sync.dma_start(
                            out=out_b[c0 * 128:(c0 + 1) * 128, 0:C],
                            in_=dbgp[:],
                        )
                    continue

                # PT = P^T (A^T unnormalized), 16 x [128,128] bf16 tiles,
                # via TensorE transpose (DMA transpose corrupts under load)
                PT = {}
                for d0 in range(G):
                    for c0 in range(G):
                        pt_ps = opsp.tile([128, 128], bf16,
                                          name=f"ptp{b}_{d0}_{c0}", tag="o")
                        nc.tensor.transpose(
                            pt_ps[:],
                            P[c0][:, d0 * 128:(d0 + 1) * 128],
                            ident_bf[:],
                        )
                        t_ = ptp.tile([128, 128], bf16, name=f"pt{b}_{d0}_{c0}",
                                      tag="pt")
                        nc.scalar.copy(t_[:], pt_ps[:])
                        PT[(d0, c0)] = t_

                if stage <= 3:
                    for d0 in range(G):
                        dbgt = resp.tile([128, C], f32, name=f"dbgt{b}_{d0}",
                                         tag="res")
                        nc.vector.tensor_copy(
                            dbgt[:, 0:128], PT[(d0, 0)][:])
                        nc.vector.tensor_copy(
                            dbgt[:, 128:256], QT[(d0, 1)][:, 0:128])
                        nc.vector.tensor_copy(
                            dbgt[:, 256:260],
                            QT[(d0, 3)][:, 100:104])
                        for c0 in range(G):
                            nc.vector.tensor_copy(
                                dbgt[:, 300 + c0:301 + c0], gR[c0][:])
                        nc.vector.tensor_copy(dbgt[:, 310:311], gb[:])
                        nc.sync.dma_start(
                            out=out_b[d0 * 128:(d0 + 1) * 128, 0:C],
                            in_=dbgt[:],
                        )
                    continue

                # mm2 + fused epilogue: out = (A@Q^T) * (gamma/Z) + x
                for n0 in range(KC):
                    for c0 in range(G):
                        o_ps = opsp.tile([128, 512], f32,
                                         name=f"o{b}_{n0}_{c0}", tag="o")
                        for d0 in range(G):
                            nc.tensor.matmul(
                                o_ps[:],
                                lhsT=PT[(d0, c0)][:],
                                rhs=QT[(d0, n0)][:],
                                start=(d0 == 0),
                                stop=(d0 == G - 1),
                            )
                        res = resp.tile([128, 512], f32,
                                        name=f"res{b}_{n0}_{c0}", tag="res")
                        nc.vector.scalar_tensor_tensor(
                            out=res[:],
                            in0=o_ps[:],
                            scalar=gR[c0][:],
                            in1=qs[(n0, c0)][:].bitcast(f32),
                            op0=AluOp.mult,
                            op1=AluOp.add,
                        )
                        nc.sync.dma_start(
                            out=out_b[c0 * 128:(c0 + 1) * 128,
                                      n0 * 512:(n0 + 1) * 512],
                            in_=res[:],
                        )

    nc.compile()
    return nc


def _get_nc():
    if "nc" not in _cache:
        _cache["nc"] = _build_nc()
    return _cache["nc"]


def kernel(x: np.ndarray, gamma: np.ndarray) -> np.ndarray:
    from concourse import bass_utils

    nc = _get_nc()

    x = np.ascontiguousarray(np.asarray(x, dtype=np.float32))
    gamma = np.asarray(gamma, dtype=np.float32).reshape(1, 1)

    q = x.reshape(B, HW, C)
    qtb = np.ascontiguousarray(
        q.astype(ml_dtypes.bfloat16).transpose(0, 2, 1))
    q0 = np.ascontiguousarray(q[0])

    in_maps = []
    for i in range(NCORES):
        idx = [i, i + NCORES]
        in_maps.append({
            "xq": np.ascontiguousarray(q[idx]),
            "x0q": q0,
            "qt16": np.ascontiguousarray(qtb[idx]),
            "gamma": gamma,
        })

    res = bass_utils.run_bass_kernel_spmd(
        nc, in_maps, core_ids=list(range(NCORES))
    )

    outp = np.empty((B, C, HW), np.float32)
    for i in range(NCORES):
        o = res.results[i]["out"]
        outp[i] = o[0]
        outp[i + NCORES] = o[1]
    return outp.reshape(B, C, 64, 64)
